# revision 15
# baseline (speedup 1.0000x reference)
"""MultiHeadTimeAttention on TRN2: LayerNorm -> QKV -> causal attention with
relative position bias -> out-proj + residual, plus per-token entropy
(mean over heads).

Sharding: data-parallel over batch; 8 cores x 2 batch items. Weights
replicated; no collectives.

Per-core dataflow (NTOK=1024 tokens = 2 x T=512), feature-major
("transposed") until the out-projection restores token-major:

  xT [D,NTOK] -> LN stats via PE-ones matmuls -> xnT (f32r)
  Q^T,K^T [feat,tok] = wqkv.T @ xnT  (bf16)
  V [tok,feat] = xnT.T @ wv          (bf16; per-head 64 cols + shared
                                      ones/zeros pad block for Z row)
  per (h, b): S^T[k,q] = K^T.T @ Q^T  (K=64 row-tiled matmuls)
  s = S^T + biasT[h] (rel-pos bias + causal -30 mask, host precomputed)
  U = exp(s) [bf16], SU = s*U [bf16]
  O'[128,512] = Vpad.T @ U : rows 0-63 head out (unnorm), row 64 = Z
  W2[1,512]   = ones.T @ SU
  attnout^T = O'[0:64] * (1/Z bcast); ent += -ln(1/Z) - W2 * (1/Z)
  final[tok,D] = attnout^T.T @ wout (+bout via K=1 aug matmul) + x
"""

import numpy as np

from concourse import bacc
from concourse.ap import AP
import concourse.mybir as mybir
import concourse.tile as tile
from concourse.bass_utils import run_bass_kernel_spmd

F32 = mybir.dt.float32
F32R = mybir.dt.float32r
BF16 = mybir.dt.bfloat16
AF = mybir.ActivationFunctionType
ALU = mybir.AluOpType

B, T, D = 16, 512, 1024
H = 16
HD = 64
MAX_SEQ = 512
LN_EPS = 1e-5
N_CORES = 8
BSH = B // N_CORES          # 2 batch items per core
NTOK = BSH * T              # 1024 tokens per core
NEG = -30.0                 # causal mask additive value

KT_D = D // 128             # 8 contraction tiles over D
KT_T = T // 128             # 4 contraction tiles over key tokens


def build_core_program():
    nc = bacc.Bacc("TRN2", target_bir_lowering=False, debug=False,
                   num_devices=N_CORES)

    xT = nc.dram_tensor("xT", [D, NTOK], F32R, kind="ExternalInput")
    x_nat = nc.dram_tensor("x_nat", [NTOK, D], F32, kind="ExternalInput")
    wqkv = nc.dram_tensor("wqkv", [KT_D, 24, 128, 128], F32R,
                          kind="ExternalInput")     # host-tiled, q cols /8
    bqkv = nc.dram_tensor("bqkv", [3 * D], F32, kind="ExternalInput")
    wout = nc.dram_tensor("wout", [KT_D, 2, 128, 512], BF16,
                          kind="ExternalInput")     # host-tiled bf16
    bout = nc.dram_tensor("bout", [D], F32R, kind="ExternalInput")
    lng = nc.dram_tensor("lng", [D], F32, kind="ExternalInput")
    lnb = nc.dram_tensor("lnb", [D], F32, kind="ExternalInput")
    biasT = nc.dram_tensor("biasT", [H, T, T], BF16, kind="ExternalInput")

    out = nc.dram_tensor("out", [NTOK, D], F32, kind="ExternalOutput")
    ent = nc.dram_tensor("ent", [BSH, T], F32, kind="ExternalOutput")

    with tile.TileContext(nc) as tc:
        with (
            tc.tile_pool(name="const", bufs=1) as const,
            tc.tile_pool(name="persist", bufs=1) as persist,
            tc.tile_pool(name="ps_mm", bufs=3, space="PSUM") as ps_mm,
            tc.tile_pool(name="ps_s", bufs=2, space="PSUM") as ps_s,
            tc.tile_pool(name="ps_o", bufs=2, space="PSUM") as ps_o,
            tc.tile_pool(name="ps_w", bufs=1, space="PSUM") as ps_w,
        ):
            # ---- constants ----
            ones_f = const.tile([128, 1], F32)
            nc.any.memset(ones_f[:], 1.0)
            ones_col = const.tile([128, 1], F32R)
            nc.vector.tensor_copy(ones_col[:], ones_f[:])
            w2ones = const.tile([128, 1], BF16)
            nc.vector.tensor_copy(w2ones[:], ones_f[:])
            onesr_f = const.tile([1, 128], F32)
            nc.any.memset(onesr_f[:], 1.0)
            ones_row = const.tile([1, 128], F32R)
            nc.vector.tensor_copy(ones_row[:], onesr_f[:])

            lng_sb = const.tile([128, KT_D], F32)
            lnb_sb = const.tile([128, KT_D], F32)
            nc.sync.dma_start(lng_sb[:], lng.ap().rearrange("(a p) -> p a", p=128))
            nc.sync.dma_start(lnb_sb[:], lnb.ap().rearrange("(a p) -> p a", p=128))
            bqk_sb = const.tile([128, 16], F32)
            nc.sync.dma_start(
                bqk_sb[:], bqkv.ap()[0:2 * D].rearrange("(a p) -> p a", p=128))
            bv_sb = const.tile([128, 2, 512], F32)
            for vch in range(2):
                nc.sync.dma_start(
                    bv_sb[:, vch, :],
                    bqkv.ap()[2 * D + vch * 512: 2 * D + (vch + 1) * 512]
                    [None, :].to_broadcast((128, 512)))
            bout_row = const.tile([1, D], F32R)
            nc.sync.dma_start(bout_row[:], bout.ap()[None, :])
            eps_sb = const.tile([1, 1], F32)
            nc.any.memset(eps_sb[:], LN_EPS)

            # ---- persistent tensors ----
            qT = persist.tile([128, 8, BSH, 512], BF16, tag="qT")
            kT = persist.tile([128, 8, BSH, 512], BF16, tag="kT")
            v_sb = persist.tile([128, BSH, KT_T, 16, 128], BF16, tag="v")
            aout = persist.tile([128, KT_D, BSH, 512], BF16, tag="aout")
            xn_sb = persist.tile([128, KT_D, NTOK], F32R, tag="xn")
            ent_acc = persist.tile([1, BSH, T], F32, tag="ent")
            nc.any.memset(ent_acc[:], 0.0)

            # ================= Phase 0: LayerNorm =================
            with tc.tile_pool(name="lnp", bufs=1) as lnp, \
                 tc.tile_pool(name="lnw", bufs=1) as lnw, \
                 tc.tile_pool(name="lnr", bufs=2) as lnr, \
                 tc.tile_pool(name="lnt", bufs=2) as lnt:
                xT_sb = lnp.tile([128, KT_D, NTOK], F32R, tag="xT")
                for kt in range(KT_D):
                    nc.sync.dma_start(xT_sb[:, kt, :],
                                      xT.ap()[kt * 128:(kt + 1) * 128, :])
                srow = lnp.tile([1, 2, NTOK], F32, tag="srow")
                # sum rows via PE-ones on xT directly
                for ch in range(NTOK // 512):
                    ps = ps_w.tile([1, 512], F32, tag="w2")
                    for kt in range(KT_D):
                        nc.tensor.matmul(
                            ps[:], ones_col[:],
                            xT_sb[:, kt, ch * 512:(ch + 1) * 512],
                            start=(kt == 0), stop=(kt == KT_D - 1))
                    nc.vector.tensor_copy(
                        srow[:, 0, ch * 512:(ch + 1) * 512], ps[:])
                # sum-of-squares: chunked squares then PE-ones
                for ch in range(NTOK // 512):
                    sq_ch = lnw.tile([128, KT_D, 512], F32R, tag="xsq")
                    for kt in range(KT_D):
                        nc.scalar.square(sq_ch[:, kt, :],
                                         xT_sb[:, kt, ch * 512:(ch + 1) * 512])
                    ps = ps_w.tile([1, 512], F32, tag="w2")
                    for kt in range(KT_D):
                        nc.tensor.matmul(ps[:], ones_col[:], sq_ch[:, kt, :],
                                         start=(kt == 0), stop=(kt == KT_D - 1))
                    nc.vector.tensor_copy(
                        srow[:, 1, ch * 512:(ch + 1) * 512], ps[:])

                musq = lnr.tile([1, NTOK], F32, tag="row")
                nc.vector.scalar_tensor_tensor(
                    musq[:], srow[:, 0, :], 1.0 / (D * D), srow[:, 0, :],
                    op0=ALU.mult, op1=ALU.mult)
                var = lnr.tile([1, NTOK], F32, tag="row")
                nc.vector.scalar_tensor_tensor(
                    var[:], srow[:, 1, :], 1.0 / D, musq[:],
                    op0=ALU.mult, op1=ALU.subtract)
                sd = lnr.tile([1, NTOK], F32, tag="row")
                nc.scalar.activation(sd[:], var[:], AF.Sqrt, bias=eps_sb[:])
                rrow = lnr.tile([1, NTOK], F32, tag="row")
                nc.vector.reciprocal(rrow[:], sd[:])
                mrow = lnr.tile([1, NTOK], F32, tag="row")
                nc.vector.scalar_tensor_tensor(
                    mrow[:], srow[:, 0, :], -1.0 / D, rrow[:],
                    op0=ALU.mult, op1=ALU.mult)

                rb = lnp.tile([128, NTOK], F32, tag="rb")
                mb = lnp.tile([128, NTOK], F32, tag="mb")
                nc.gpsimd.partition_broadcast(rb[:], rrow[:])
                nc.gpsimd.partition_broadcast(mb[:], mrow[:])

                for kt in range(KT_D):
                    t1 = lnt.tile([128, NTOK], F32, tag="lnt1")
                    nc.vector.tensor_tensor(t1[:], xT_sb[:, kt, :].bitcast(F32),
                                            rb[:], op=ALU.mult)
                    nc.vector.tensor_tensor(t1[:], t1[:], mb[:], op=ALU.add)
                    nc.vector.tensor_scalar(
                        xn_sb[:, kt, :], t1[:],
                        lng_sb[:, kt:kt + 1], lnb_sb[:, kt:kt + 1],
                        op0=ALU.mult, op1=ALU.add)

            # ================= Phase 1: QKV =================
            with (
                tc.tile_pool(name="wqp", bufs=18) as wqp,
                tc.tile_pool(name="wvp", bufs=10) as wvp,
            ):
                for m in range(16):
                    wtiles = []
                    for kt in range(KT_D):
                        wt = wqp.tile([128, 128], F32R, tag="wq")
                        nc.sync.dma_start(wt[:], wqkv.ap()[kt, m, :, :])
                        wtiles.append(wt)
                    for b in range(BSH):
                        ps = ps_mm.tile([128, 512], F32, tag="mm")
                        for kt in range(KT_D):
                            nc.tensor.matmul(
                                ps[:], wtiles[kt][:],
                                xn_sb[:, kt, b * 512:(b + 1) * 512],
                                start=(kt == 0), stop=(kt == KT_D - 1))
                        dst = qT if m < 8 else kT
                        nc.vector.tensor_scalar(
                            dst[:, m % 8, b, :], ps[:],
                            bqk_sb[:, m:m + 1], None, op0=ALU.add)

                # V pad init: col 64 of each head block = 1, cols 65.. = 0
                nc.any.memset(v_sb[:, :, :, :, 64:128], 0.0)
                nc.any.memset(v_sb[:, :, :, :, 64:65], 1.0)
                for vch in range(2):
                    wvtiles = []
                    for kt in range(KT_D):
                        wt = wvp.tile([128, 512], F32R, tag="wv")
                        nc.sync.dma_start(
                            wt[:].rearrange("p (m f) -> p m f", f=128),
                            wqkv.ap()[kt, 16 + vch * 4:16 + (vch + 1) * 4, :, :]
                            .rearrange("m p f -> p m f"))
                        wvtiles.append(wt)
                    for b in range(BSH):
                        for vt in range(KT_T):
                            ps = ps_mm.tile([128, 512], F32, tag="mm")
                            mt = b * KT_T + vt
                            for kt in range(KT_D):
                                nc.tensor.matmul(
                                    ps[:],
                                    xn_sb[:, kt, mt * 128:(mt + 1) * 128],
                                    wvtiles[kt][:],
                                    start=(kt == 0), stop=(kt == KT_D - 1))
                            nc.vector.scalar_tensor_tensor(
                                v_sb[:, b, vt, vch * 8:(vch + 1) * 8, 0:64],
                                ps[:].rearrange("p (h f) -> p h f", f=64),
                                0.0,
                                bv_sb[:, vch, :].rearrange(
                                    "p (h f) -> p h f", f=64),
                                op0=ALU.add, op1=ALU.add)

            # ================= Phase 2: attention =================
            with tc.tile_pool(name="atp", bufs=2) as atp, \
                 tc.tile_pool(name="bp", bufs=2) as bp:
                for h in range(H):
                    hp, lo = h // 2, (h % 2) * 64
                    bt_sb = bp.tile([128, KT_T, 512], BF16, tag="bias")
                    for kt in range(KT_T):
                        nc.sync.dma_start(
                            bt_sb[:, kt, :],
                            biasT.ap()[h, kt * 128:(kt + 1) * 128, :])
                    for b in range(BSH):
                        s_sb = atp.tile([128, KT_T, 512], F32, tag="s")
                        u_sb = atp.tile([128, KT_T, 512], BF16, tag="u")
                        su_sb = atp.tile([128, KT_T, 512], BF16, tag="su")
                        for kt in range(KT_T):
                            sps = ps_s.tile([128, 512], F32, tag="smm")
                            nc.tensor.matmul(
                                sps[:],
                                kT[lo:lo + 64, hp, b, kt * 128:(kt + 1) * 128],
                                qT[lo:lo + 64, hp, b, :],
                                start=True, stop=True,
                                tile_position=(lo, 0))
                            nc.vector.tensor_tensor(
                                s_sb[:, kt, :], sps[:], bt_sb[:, kt, :],
                                op=ALU.add)
                            nc.scalar.activation(u_sb[:, kt, :], s_sb[:, kt, :],
                                                 AF.Exp)
                            nc.vector.tensor_tensor(
                                su_sb[:, kt, :], s_sb[:, kt, :],
                                u_sb[:, kt, :], op=ALU.mult)

                        ops = ps_o.tile([128, 512], F32, tag="opv")
                        w2ps = ps_w.tile([1, 512], F32, tag="w2")
                        for kt in range(KT_T):
                            nc.tensor.matmul(
                                ops[:], v_sb[:, b, kt, h, :],
                                u_sb[:, kt, :],
                                start=(kt == 0), stop=(kt == KT_T - 1))
                            nc.tensor.matmul(
                                w2ps[:], w2ones[:], su_sb[:, kt, :],
                                start=(kt == 0), stop=(kt == KT_T - 1))

                        zr = atp.tile([1, 512], F32, tag="zr")
                        nc.vector.reciprocal(zr[:], ops[64:65, :])
                        zb = atp.tile([64, 512], F32, tag="zb")
                        nc.gpsimd.partition_broadcast(zb[:], zr[:])
                        nc.vector.tensor_tensor(
                            aout[lo:lo + 64, h // 2, b, :],
                            ops[0:64, :], zb[:], op=ALU.mult)

                        lnzr = atp.tile([1, 512], F32, tag="lnzr")
                        nc.scalar.activation(lnzr[:], zr[:], AF.Ln)
                        t = atp.tile([1, 512], F32, tag="entt")
                        nc.vector.tensor_tensor(t[:], w2ps[:], zr[:], op=ALU.mult)
                        nc.vector.tensor_tensor(t[:], t[:], lnzr[:], op=ALU.add)
                        nc.vector.tensor_tensor(
                            ent_acc[:, b, :], ent_acc[:, b, :], t[:],
                            op=ALU.subtract)

                ent_out = atp.tile([1, BSH, T], F32, tag="ent_out")
                nc.vector.tensor_scalar(ent_out[:], ent_acc[:], 1.0 / H, None,
                                        op0=ALU.mult)
                for b in range(BSH):
                    nc.sync.dma_start(ent.ap()[b:b + 1, :], ent_out[:, b, :])

            # ================= Phase 3: out projection =================
            with tc.tile_pool(name="otp", bufs=3) as otp, \
                 tc.tile_pool(name="wop", bufs=10) as wop, \
                 tc.tile_pool(name="xsp", bufs=1) as xsp:
                x_sb = xsp.tile([128, KT_D, D], F32, tag="xnat")
                for mt in range(KT_D):
                    nc.sync.dma_start(x_sb[:, mt, :],
                                      x_nat.ap()[mt * 128:(mt + 1) * 128, :])
                for nch in range(2):
                    wotiles = []
                    for ft in range(KT_D):
                        wt = wop.tile([128, 512], BF16, tag="wo")
                        nc.sync.dma_start(wt[:], wout.ap()[ft, nch, :, :])
                        wotiles.append(wt)
                    for b in range(BSH):
                        for mt in range(4):
                            gmt = b * 4 + mt
                            ps = ps_mm.tile([128, 512], F32, tag="mm")
                            for ft in range(KT_D):
                                nc.tensor.matmul(
                                    ps[:],
                                    aout[:, ft, b, mt * 128:(mt + 1) * 128],
                                    wotiles[ft][:],
                                    start=(ft == 0), stop=False)
                            nc.tensor.matmul(
                                ps[:], ones_row[:],
                                bout_row[:, nch * 512:(nch + 1) * 512],
                                start=False, stop=True)
                            o_sb = otp.tile([128, 512], F32, tag="osb")
                            nc.vector.tensor_tensor(
                                o_sb[:], ps[:],
                                x_sb[:, gmt, nch * 512:(nch + 1) * 512],
                                op=ALU.add)
                            nc.sync.dma_start(
                                out.ap()[gmt * 128:(gmt + 1) * 128,
                                         nch * 512:(nch + 1) * 512], o_sb[:])

    nc.compile()
    return nc


_NC_CACHE = None


def _get_program():
    global _NC_CACHE
    if _NC_CACHE is None:
        _NC_CACHE = build_core_program()
    return _NC_CACHE


def _host_prep(x, qkv_w, qkv_b, out_w, out_b, ln_g, ln_b, rel_bias):
    import ml_dtypes
    x = np.asarray(x, dtype=np.float32)
    qkv_w = np.asarray(qkv_w, dtype=np.float32)
    qkv_b = np.asarray(qkv_b, dtype=np.float32)
    out_w = np.asarray(out_w, dtype=np.float32)
    out_b = np.asarray(out_b, dtype=np.float32)
    ln_g = np.asarray(ln_g, dtype=np.float32)
    ln_b = np.asarray(ln_b, dtype=np.float32)
    rel_bias = np.asarray(rel_bias, dtype=np.float32)

    wq = qkv_w.copy()
    wq[:, :D] *= np.float32(1.0 / np.sqrt(HD))
    bq = qkv_b.copy()
    bq[:D] *= np.float32(1.0 / np.sqrt(HD))
    wqkv_t = np.ascontiguousarray(
        wq.reshape(KT_D, 128, 24, 128).transpose(0, 2, 1, 3))
    wout_t = np.ascontiguousarray(
        out_w.reshape(KT_D, 128, 2, 512).transpose(0, 2, 1, 3)
    ).astype(ml_dtypes.bfloat16)

    kk = np.arange(T)[:, None]
    qq = np.arange(T)[None, :]
    idx = np.clip(kk - qq + (MAX_SEQ - 1), 0, 2 * MAX_SEQ - 2)
    biasT = rel_bias[:, idx]                      # [H, k, q]
    biasT = np.where((kk > qq)[None], np.float32(NEG), biasT)
    biasT = np.ascontiguousarray(biasT.astype(ml_dtypes.bfloat16))

    in_maps = []
    for c in range(N_CORES):
        xs = x[c * BSH:(c + 1) * BSH].reshape(NTOK, D)
        in_maps.append({
            "xT": np.ascontiguousarray(xs.T),
            "x_nat": np.ascontiguousarray(xs),
            "wqkv": wqkv_t,
            "bqkv": bq,
            "wout": wout_t,
            "bout": out_b,
            "lng": ln_g,
            "lnb": ln_b,
            "biasT": biasT,
        })
    return in_maps


def kernel(x, qkv_w, qkv_b, out_w, out_b, ln_g, ln_b, rel_bias, **kw):
    nc = _get_program()
    in_maps = _host_prep(x, qkv_w, qkv_b, out_w, out_b, ln_g, ln_b, rel_bias)
    res = run_bass_kernel_spmd(nc, in_maps, core_ids=list(range(N_CORES)))
    outs = [r["out"].reshape(BSH, T, D) for r in res.results]
    ents = [r["ent"] for r in res.results]
    return np.concatenate(outs, 0), np.concatenate(ents, 0)
